# revision 16
# baseline (speedup 1.0000x reference)
"""ContextMultiHeadAttn Trainium2 Bass kernel (batch-parallel over 8 cores).

Shapes (hardcoded): Q_LEN=KV_LEN=1024, BATCH=8, D_MODEL=1024, N_HEAD=16,
D_HEAD=64, EPS=1e-5.  reference: pre-LN -> q/kv proj -> per-head scaled
dot-product attention with key mask (masked j -> -1e4) -> softmax over keys
-> out proj -> residual with normalized h.

Sharding: one batch element per NeuronCore (BATCH == 8 == n_cores), no
collectives.  Inside each core everything runs in float32 except the
matmuls, which use float32r (TF32-like; 1 PE cycle/row at N>=256, measured
~1e-4 rel err).

Key tricks:
 - softmax without max-subtraction (scores ~N(0,1); exp can't overflow)
 - key mask applied multiplicatively to v rows and to a ones-column
   appended per head (column 64), whose AV-matmul row gives the masked
   softmax denominator: where(mask,-1e4,s)->softmax == exp(s)*!m / sum(exp(s)*!m)
 - scores computed transposed (ST[j,i]) so exp output E[j,i] feeds the AV
   matmul directly as the moving operand; the attention output comes out
   as AT[nd,i], which is exactly the stationary operand the o-projection
   needs, so no probability/attention transposes are ever required.
 - denominator reciprocal broadcast across partitions via gpsimd.

i (query) dim processed in two blocks of 512 to fit SBUF; hn is bounced
through a DRAM scratch between LayerNorm and the residual add.
"""

import sys
import types

import numpy as np

import concourse.bacc as bacc
import concourse.bass as bass
import concourse.mybir as mybir
import concourse.tile as tile
from concourse.bass_utils import run_bass_kernel_spmd
from concourse.masks import make_identity


def _ensure_axon_hooks():
    """bass_utils imports antenv.axon_hooks when trace=True under axon; some
    images lack that module. Install a functional shim (wired to the axon
    NTFF profiler when available) so tracing works and never crashes."""
    try:
        import antenv.axon_hooks  # noqa: F401
        return
    except ImportError:
        pass
    try:
        import antenv
    except ImportError:
        return
    mod = types.ModuleType("antenv.axon_hooks")
    mod._hook = None
    mod.set_axon_ntff_profile_hook = lambda h: setattr(mod, "_hook", h)
    mod.get_axon_ntff_profile_hook = lambda: mod._hook
    sys.modules["antenv.axon_hooks"] = mod
    antenv.axon_hooks = mod
    try:
        from trn_agent_boot.trn_boot import _ntff_profile_via_ctypes

        hook = _ntff_profile_via_ctypes("/opt/axon/libaxon_pjrt.so")
        if hook is not None:
            mod.set_axon_ntff_profile_hook(hook)
    except Exception:
        pass


_ensure_axon_hooks()

F32 = mybir.dt.float32
F32R = mybir.dt.float32r
AF = mybir.ActivationFunctionType
ALU = mybir.AluOpType
AX = mybir.AxisListType

P = 128
Q_LEN, KV_LEN, BATCH = 1024, 1024, 8
D_MODEL, N_HEAD, D_HEAD = 1024, 16, 64
ND = N_HEAD * D_HEAD
EPS = 1e-5
SCALE = 1.0 / 8.0  # 1/sqrt(D_HEAD)
IB = 512  # i-block
N_IB = Q_LEN // IB
DT = D_MODEL // P  # 8
JT = KV_LEN // P  # 8
CT = ND // P  # 8
IT = IB // P  # 4 i-tiles per block
HV = D_HEAD + 1  # 65


def build(affine: bool, upto: int = 7):
    nc = bacc.Bacc()
    h_d = nc.dram_tensor("h", [Q_LEN, D_MODEL], F32, kind="ExternalInput")
    c_d = nc.dram_tensor("c", [KV_LEN, D_MODEL], F32, kind="ExternalInput")
    mbar_d = nc.dram_tensor("mbar", [P, JT], F32, kind="ExternalInput")
    # host pre-tiled: wqk_t[s, ct, p, dt, cc] (s=0: Wq, s=1: Wkv_k)
    wqk_d = nc.dram_tensor("WqkT", [2, CT, P, DT, P], F32R, kind="ExternalInput")
    wv_d = nc.dram_tensor("Wv", [D_MODEL, ND], F32R, kind="ExternalInput")
    wo_d = nc.dram_tensor("Wo", [ND, D_MODEL], F32R, kind="ExternalInput")
    if affine:
        gamma_d = nc.dram_tensor("gammaT", [P, DT], F32, kind="ExternalInput")
        beta_d = nc.dram_tensor("betaT", [P, DT], F32, kind="ExternalInput")
    out_d = nc.dram_tensor("out", [Q_LEN, D_MODEL], F32, kind="ExternalOutput")

    with tile.TileContext(nc) as tc:
        with (
            tc.tile_pool(name="const", bufs=1) as constp,
            tc.tile_pool(name="stage", bufs=3) as stage,
            tc.tile_pool(name="wct", bufs=3) as wctp,
            tc.tile_pool(name="wrow", bufs=3) as wrowp,
            tc.tile_pool(name="halfp", bufs=2) as halfp,
            tc.tile_pool(name="bigsb", bufs=1) as bigsb,
            tc.tile_pool(name="stats", bufs=2) as statsp,
            tc.tile_pool(name="rr", bufs=2) as rrp,
            tc.tile_pool(name="ppbig", bufs=2, space="PSUM") as ppbig,
            tc.tile_pool(name="ppsm", bufs=4, space="PSUM") as ppsm,
            tc.tile_pool(name="dram", bufs=1, space="DRAM") as dramp,
        ):
            hn_d = dramp.tile([Q_LEN, D_MODEL], F32, tag="hn_scratch")
            cst = constp.tile([P, P + JT + 1], F32, tag="const")
            ident = cst[:, 0:P]
            mbar = cst[:, P : P + JT]
            eps_t = cst[:, P + JT : P + JT + 1]
            make_identity(nc, ident)
            nc.sync.dma_start(mbar, mbar_d[:, :])
            nc.vector.memset(eps_t, EPS)
            if affine:
                gb = constp.tile([P, 2 * DT], F32, tag="gb")
                gammaT = gb[:, 0:DT]
                betaT = gb[:, DT : 2 * DT]
                nc.sync.dma_start(gammaT, gamma_d[:, :])
                nc.sync.dma_start(betaT, beta_d[:, :])
                gbb = constp.tile([P, 2 * D_MODEL], F32, tag="gbb")
                gamma_b = gbb[:, 0:D_MODEL]
                beta_b = gbb[:, D_MODEL:]
                nc.sync.dma_start(
                    gamma_b,
                    gamma_d[:, :].rearrange("p o -> (o p)")[None, :].to_broadcast((P, D_MODEL)),
                )
                nc.sync.dma_start(
                    beta_b,
                    beta_d[:, :].rearrange("p o -> (o p)")[None, :].to_broadcast((P, D_MODEL)),
                )

            kT = bigsb.tile([P, CT, KV_LEN], F32R, tag="kT", name="kT") if upto >= 2 else None
            vv = bigsb.tile([P, JT, N_HEAD, HV], F32R, tag="vv", name="vv") if upto >= 3 else None
            qT = bigsb.tile([P, CT, IB], F32R, tag="qT", name="qT") if upto in (5, 6, 7) else None
            AT = bigsb.tile([P, CT, IB], F32R, tag="AT", name="AT") if upto in (6, 7) else None

            # ---------- A: c -> cT (PE transposes) ----------
            cT = [halfp.tile([P, DT // 2, KV_LEN], F32R, tag="half16", name=f"cT{i}") for i in range(2)]

            def cT_ap(dt):
                return cT[dt // 4][:, dt % 4, :]

            for jt in range(JT):
                c_t = stage.tile([P, D_MODEL], F32, tag="stage")
                nc.sync.dma_start(c_t[:], c_d[jt * P : (jt + 1) * P, :])
                for dt in range(DT):
                    pt = ppsm.tile([P, IB], F32, tag="sm")
                    nc.tensor.transpose(pt[:, :P], c_t[:, dt * P : (dt + 1) * P], ident)
                    nc.vector.tensor_copy(
                        cT[dt // 4][:, dt % 4, jt * P : (jt + 1) * P], pt[:, :P]
                    )

            # ---------- B: kT projection ----------
            for ct in range(CT if upto >= 2 else 0):
                wk_t = wctp.tile([P, DT, P], F32R, tag="wct")
                nc.sync.dma_start(wk_t[:], wqk_d[1, ct])
                pk = ppbig.tile([P, KV_LEN], F32, tag="big")
                for dt in range(DT):
                    for hf in range(2):
                        nc.tensor.matmul(
                            pk[:, hf * 512 : (hf + 1) * 512],
                            wk_t[:, dt, :],
                            cT_ap(dt)[:, hf * 512 : (hf + 1) * 512],
                            start=(dt == 0),
                            stop=(dt == DT - 1),
                        )
                nc.vector.tensor_copy(kT[:, ct, :], pk[:])

            # ---------- C: v projection (PSUM-flipped; masked into vv) ----------
            for hf in range(2 if upto >= 3 else 0):  # output column halves: heads 0-7 | 8-15
                pb = [ppbig.tile([P, KV_LEN], F32, tag="big", name=f"vpb{hf}_{i}") for i in range(2)]
                ps4 = [ppsm.tile([P, IB], F32, tag="sm", name=f"vps{hf}_{i}") for i in range(4)]

                def vreg(jt):
                    if jt < 4:
                        return pb[jt // 2][:, (jt % 2) * 512 : (jt % 2 + 1) * 512]
                    return ps4[jt - 4][:]

                for dt in range(DT):
                    wv_t = wrowp.tile([P, D_MODEL], F32R, tag="wrow")
                    nc.sync.dma_start(
                        wv_t[:, 0:512],
                        wv_d[dt * P : (dt + 1) * P, hf * 512 : (hf + 1) * 512],
                    )
                    for jt in range(JT):
                        nc.tensor.matmul(
                            vreg(jt),
                            cT_ap(dt)[:, jt * P : (jt + 1) * P],
                            wv_t[:, 0:512],
                            start=(dt == 0),
                            stop=(dt == DT - 1),
                        )
                for jt in range(JT):
                    nc.vector.tensor_scalar(
                        vv[:, jt, hf * 8 : (hf + 1) * 8, 0:D_HEAD],
                        vreg(jt).rearrange("p (n e) -> p n e", e=D_HEAD),
                        mbar[:, jt : jt + 1],
                        None,
                        ALU.mult,
                    )
                    nc.vector.tensor_copy(
                        vv[:, jt, hf * 8 : (hf + 1) * 8, D_HEAD],
                        mbar[:, jt : jt + 1].to_broadcast((P, 8)),
                    )

            # ---------- per-i-block ----------
            hn_dma_on = upto not in (40, 42)
            hnT_on = upto not in (40, 41)
            for ib in range(N_IB if upto == 7 else (1 if upto >= 4 else 0)):
                i0 = ib * IB

                # LayerNorm
                for it in range(IT):
                    h_t = stage.tile([P, D_MODEL], F32, tag="stage")
                    nc.sync.dma_start(h_t[:], h_d[i0 + it * P : i0 + (it + 1) * P, :])
                    st = statsp.tile([P, 8], F32, tag="st")
                    nc.vector.reduce_sum(st[:, 0:1], h_t[:], axis=AX.X, negate=True)
                    nc.vector.tensor_scalar_mul(st[:, 1:2], st[:, 0:1], 1.0 / D_MODEL)
                    cen = stage.tile([P, D_MODEL], F32, tag="stage")
                    nc.vector.tensor_scalar_add(cen[:], h_t[:], st[:, 1:2])
                    nc.vector.tensor_tensor(h_t[:], cen[:], cen[:], ALU.mult)
                    nc.vector.reduce_sum(st[:, 2:3], h_t[:], axis=AX.X)
                    nc.scalar.activation(st[:, 3:4], st[:, 2:3], AF.Sqrt, bias=eps_t, scale=1.0 / D_MODEL)
                    nc.vector.reciprocal(st[:, 4:5], st[:, 3:4])
                    nc.vector.tensor_scalar_mul(h_t[:], cen[:], st[:, 4:5])  # h_t := hn
                    if hn_dma_on:
                        nc.sync.dma_start(hn_d[i0 + it * P : i0 + (it + 1) * P, :], h_t[:])
                    if upto in (40, 41) and it == 0:
                        nc.sync.dma_start(out_d[0:P, :], h_t[:])

                    # hnT transposes for this i-tile (fused gamma/beta if affine)
                    if it == 0 and hnT_on:
                        hnT = halfp.tile([P, DT, IB], F32R, tag="half16")
                    for dt in range(DT if hnT_on else 0):
                        pt = ppsm.tile([P, IB], F32, tag="sm")
                        nc.tensor.transpose(pt[:, :P], h_t[:, dt * P : (dt + 1) * P], ident)
                        if affine:
                            nc.vector.tensor_scalar(
                                hnT[:, dt, it * P : (it + 1) * P],
                                pt[:, :P],
                                gammaT[:, dt : dt + 1],
                                betaT[:, dt : dt + 1],
                                ALU.mult,
                                ALU.add,
                            )
                        else:
                            nc.vector.tensor_copy(hnT[:, dt, it * P : (it + 1) * P], pt[:, :P])

                # qT projection
                for ct in range(CT if upto in (5, 6, 7) else 0):
                    wq_t = wctp.tile([P, DT, P], F32R, tag="wct")
                    nc.sync.dma_start(wq_t[:], wqk_d[0, ct])
                    pq = ppsm.tile([P, IB], F32, tag="sm")
                    for dt in range(DT):
                        nc.tensor.matmul(
                            pq[:],
                            wq_t[:, dt, :],
                            hnT[:, dt, :],
                            start=(dt == 0),
                            stop=(dt == DT - 1),
                        )
                    nc.vector.tensor_copy(qT[:, ct, :], pq[:])

                # heads
                for n in range(N_HEAD if upto in (6, 7) else 0):
                    prow = (n % 2) * D_HEAD
                    ct_n = n // 2
                    E = halfp.tile([P, JT, IB], F32R, tag="half16")
                    for jp in range(JT // 2):
                        ps = ppbig.tile([P, 2 * IB], F32, tag="big")
                        for sub in range(2):
                            jt = jp * 2 + sub
                            nc.tensor.matmul(
                                ps[:, sub * IB : (sub + 1) * IB],
                                kT[prow : prow + D_HEAD, ct_n, jt * P : (jt + 1) * P],
                                qT[prow : prow + D_HEAD, ct_n, :],
                                start=True,
                                stop=True,
                            )
                        nc.scalar.activation(
                            E[:, jp * 2 : jp * 2 + 2, :].rearrange("p a b -> p (a b)"),
                            ps[:],
                            AF.Exp,
                            scale=SCALE,
                        )
                    pa = ppsm.tile([P, IB], F32, tag="sm")
                    for jt in range(JT):
                        nc.tensor.matmul(
                            pa[0:HV, :],
                            vv[:, jt, n, :],
                            E[:, jt, :],
                            start=(jt == 0),
                            stop=(jt == JT - 1),
                        )
                    recip = rrp.tile([1, IB], F32, tag="rr")
                    nc.vector.reciprocal(recip[:], pa[D_HEAD : D_HEAD + 1, :])
                    rb = rrp.tile([D_HEAD, IB], F32, tag="rr")
                    nc.gpsimd.partition_broadcast(rb[:], recip[:])
                    nc.vector.tensor_tensor(
                        AT[prow : prow + D_HEAD, ct_n, :],
                        pa[0:D_HEAD, :],
                        rb[:],
                        ALU.mult,
                    )

                # o-projection (kt-outer) + residual
                if upto != 7:
                    continue
                pbo = [ppbig.tile([P, D_MODEL], F32, tag="big", name=f"opb{ib}_{i}") for i in range(2)]
                pso = [ppsm.tile([P, IB], F32, tag="sm", name=f"ops{ib}_{i}") for i in range(4)]

                def oreg(ic, hf):
                    if ic < 2:
                        return pbo[ic][:, hf * 512 : (hf + 1) * 512]
                    return pso[(ic - 2) * 2 + hf][:]

                for kt in range(CT):
                    wo_t = wrowp.tile([P, D_MODEL], F32R, tag="wrow")
                    nc.sync.dma_start(wo_t[:], wo_d[kt * P : (kt + 1) * P, :])
                    for ic in range(IT):
                        for hf in range(2):
                            nc.tensor.matmul(
                                oreg(ic, hf),
                                AT[:, kt, ic * P : (ic + 1) * P],
                                wo_t[:, hf * 512 : (hf + 1) * 512],
                                start=(kt == 0),
                                stop=(kt == CT - 1),
                            )
                for ic in range(IT):
                    hn_t = stage.tile([P, D_MODEL], F32, tag="stage")
                    nc.sync.dma_start(hn_t[:], hn_d[i0 + ic * P : i0 + (ic + 1) * P, :])
                    o_t = stage.tile([P, D_MODEL], F32, tag="stage")
                    if affine:
                        nc.vector.tensor_tensor(o_t[:], hn_t[:], gamma_b, ALU.mult)
                        nc.vector.tensor_tensor(o_t[:], o_t[:], beta_b, ALU.add)
                        for hf in range(2):
                            nc.vector.tensor_tensor(
                                o_t[:, hf * 512 : (hf + 1) * 512],
                                o_t[:, hf * 512 : (hf + 1) * 512],
                                oreg(ic, hf),
                                ALU.add,
                            )
                    else:
                        for hf in range(2):
                            nc.vector.tensor_tensor(
                                o_t[:, hf * 512 : (hf + 1) * 512],
                                oreg(ic, hf),
                                hn_t[:, hf * 512 : (hf + 1) * 512],
                                ALU.add,
                            )
                    nc.sync.dma_start(out_d[i0 + ic * P : i0 + (ic + 1) * P, :], o_t[:])

            if upto == 1:
                nc.sync.dma_start(out_d[0:P, :].bitcast(F32R), cT[0][:, 0, :])
            elif upto == 2:
                nc.sync.dma_start(out_d[0:P, :].bitcast(F32R), kT[:, 0, :])
            elif upto == 3:
                nc.sync.dma_start(
                    out_d[0:P, :].bitcast(F32R),
                    vv[:, 0, :, :].rearrange("p a b -> p (a b)")[:, 0:D_MODEL],
                )
            elif upto in (4, 42):
                nc.sync.dma_start(out_d[0:P, 0:IB].bitcast(F32R), hnT[:, 0, :])
            elif upto == 5:
                nc.sync.dma_start(out_d[0:P, 0:IB].bitcast(F32R), qT[:, 0, :])
            elif upto == 6:
                nc.sync.dma_start(out_d[0:P, 0:IB].bitcast(F32R), AT[:, 0, :])

    nc.compile()
    return nc


_CACHE: dict = {}


def _get_nc(affine: bool):
    if affine not in _CACHE:
        _CACHE[affine] = build(affine)
    return _CACHE[affine]


def kernel(h, c, attn_mask, Wq, Wkv, Wo, gamma, beta):
    h = np.asarray(h, dtype=np.float32)
    c = np.asarray(c, dtype=np.float32)
    attn_mask = np.asarray(attn_mask)
    Wq = np.asarray(Wq, dtype=np.float32)
    Wkv = np.asarray(Wkv, dtype=np.float32)
    Wo = np.asarray(Wo, dtype=np.float32)
    gamma = np.asarray(gamma, dtype=np.float32)
    beta = np.asarray(beta, dtype=np.float32)

    affine = not (np.all(gamma == 1.0) and np.all(beta == 0.0))
    nc = _get_nc(affine)

    # column-tiled stationary layout [s, ct, p, dt, cc] for Wq / Wkv_k
    def ctile(w):  # w: [D_MODEL, ND]
        return w.reshape(DT, P, CT, P).transpose(2, 1, 0, 3)

    wqk = np.ascontiguousarray(
        np.stack([ctile(Wq), ctile(Wkv[:, 0:ND])], axis=0)
    )
    wv = np.ascontiguousarray(Wkv[:, ND : 2 * ND])
    mbar_all = (~attn_mask.astype(bool)).astype(np.float32)  # [KV_LEN, BATCH]

    in_maps = []
    for b in range(BATCH):
        m = {
            "h": np.ascontiguousarray(h[:, b, :]),
            "c": np.ascontiguousarray(c[:, b, :]),
            "mbar": np.ascontiguousarray(mbar_all[:, b].reshape(JT, P).T),
            "WqkT": wqk,
            "Wv": wv,
            "Wo": Wo,
        }
        if affine:
            m["gammaT"] = np.ascontiguousarray(gamma.reshape(DT, P).T)
            m["betaT"] = np.ascontiguousarray(beta.reshape(DT, P).T)
        in_maps.append(m)

    res = run_bass_kernel_spmd(nc, in_maps, core_ids=list(range(BATCH)))
    global LAST_EXEC_NS, LAST_TRACE
    if res.exec_time_ns is not None:
        LAST_EXEC_NS = res.exec_time_ns
    if res.instructions_and_trace is not None:
        LAST_TRACE = res.instructions_and_trace[1]
    out = np.stack([res.results[b]["out"] for b in range(BATCH)], axis=1)
    return np.ascontiguousarray(out.astype(np.float32))


LAST_EXEC_NS = None
LAST_TRACE = None
